# revision 21
# baseline (speedup 1.0000x reference)
"""Dynamic Depthwise Conv (DDC) module on 8 Trainium2 NeuronCores.

Strategy (data-parallel over batch, 4 samples/core):
  - Depthwise 3x3 conv on TensorE as 9 "diagonal matmuls" per 128-channel
    tile: lhsT = diag(per-channel tap weight) [128x128] bf16, rhs = a
    zero-column-padded bf16 image tile, accumulated in PSUM (fp32).
  - Kernel/bias generation branches (pooling -> 1x1 convs -> BN -> GELU ->
    1x1 conv -> softmax-over-2 == sigmoid of difference) computed on-chip:
    pooling block sums on VectorE, small matmuls on TensorE in bf16 batched
    over sample groups [1, 3] (group 0 unblocks the conv pipeline early,
    group 1 amortizes weight loads), exact GELU via Erf on ScalarE (all
    transcendentals live in the single `sigmoid_and_others` ACT table set).
  - BN scale and the 1/256 / 1/2304 pooling-mean factors are folded into
    host-precomputed transposed weight matrices (layout prep only).
  - f32->bf16 padded-image casts run on GpSimd to keep VectorE for pooling.
  - PSUM->SBUF copyback with per-channel bias add on ScalarE (Identity+bias).
"""

import numpy as np
import ml_dtypes
from contextlib import ExitStack

import concourse.bass as bass
import concourse.bacc as bacc
import concourse.tile as tile
import concourse.mybir as mybir
from concourse.bass_utils import run_bass_kernel_spmd

f32 = mybir.dt.float32
bf16 = mybir.dt.bfloat16
AF = mybir.ActivationFunctionType
AL = mybir.AluOpType

N_CORES = 8
B, C, H, W = 32, 512, 48, 48
BL = B // N_CORES          # samples per core
NT = C // 128              # channel tiles per sample
HW = H * W                 # 2304
WP = W + 2                 # padded row length (2 zero cols at end)
PADN = H * WP + 2          # 1 guard elem at 0, data at 1..; 2402
Cr = 256
EPS = 1e-5
INV_SQRT2 = 0.7071067811865476
GROUPS = [[0], [1, 2, 3]]  # sample batching for the generator branches

# tap order: dy=0 taps first so the first matmul per PSUM bank covers all rows
TAPS = [(0, 0), (0, -1), (0, 1),
        (-1, -1), (-1, 0), (-1, 1),
        (1, -1), (1, 0), (1, 1)]
N_XPAD_BUFS = 14

# vecs tile column layout
V_PBETA, V_PBETA_S, V_QBETA, V_QBETA_S = 0, 2, 4, 6
V_PB2, V_QB2D, V_DWB1, V_DWBD = 8, 12, 16, 20


def _emit_kernel(nc, t_in, t_out):
    x = t_in["x"].ap()            # [BL, C, H, W] f32
    y = t_out["y"].ap()           # [BL, C, H, W] f32

    with tile.TileContext(nc) as tc, ExitStack() as ctx:
        singles = ctx.enter_context(tc.tile_pool(name="singles", bufs=1))
        xf32_pool = ctx.enter_context(tc.tile_pool(name="xf32", bufs=4))
        xpad_pool = ctx.enter_context(tc.tile_pool(name="xpad", bufs=1))
        cb_pool = ctx.enter_context(tc.tile_pool(name="cb", bufs=4))
        bs_pool = ctx.enter_context(tc.tile_pool(name="bs", bufs=2 * NT))
        sm_pool = ctx.enter_context(tc.tile_pool(name="smalls", bufs=6))
        wt_pool = ctx.enter_context(tc.tile_pool(name="wt", bufs=2 * BL * NT))
        diag_pool = ctx.enter_context(tc.tile_pool(name="diag", bufs=4))
        ysb_pool = ctx.enter_context(tc.tile_pool(name="ysb", bufs=2))
        psum_small = ctx.enter_context(
            tc.tile_pool(name="ps_small", bufs=2, space="PSUM"))
        psum_conv = ctx.enter_context(
            tc.tile_pool(name="ps_conv", bufs=2, space="PSUM"))

        bs_of = {g: [bs_pool.tile([128, 9 * len(grp)], f32,
                                  name=f"bs_g{g}_{t}")
                     for t in range(NT)]
                 for g, grp in enumerate(GROUPS)}

        # prefetch sample-0 image tiles before the parameter DMAs so pooling
        # starts as early as possible
        xf_prefetch = {}
        for t in range(NT):
            xf = xf32_pool.tile([128, HW], f32, name="xf")
            nc.sync.dma_start(
                out=xf,
                in_=x[0, 128 * t:128 * (t + 1)].rearrange("c h w -> c (h w)"))
            xf_prefetch[t] = xf

        def load_param(name, sbuf_shape, src_ap, dtype=f32):
            t = singles.tile(sbuf_shape, dtype, name=name)
            nc.sync.dma_start(out=t, in_=src_ap)
            return t

        # big matrices in bf16 (matmul weights), vectors consolidated
        pw1t = load_param("pw1t", [128, NT, Cr],
                          t_in["pw1t"].ap().rearrange("(k p) m -> p k m", p=128),
                          bf16)
        pw2t = load_param("pw2t", [128, 2, C],
                          t_in["pw2t"].ap().rearrange("(k p) m -> p k m", p=128),
                          bf16)
        qw1t = load_param("qw1t", [128, NT, Cr],
                          t_in["qw1t"].ap().rearrange("(k p) m -> p k m", p=128),
                          bf16)
        qw2dt = load_param("qw2dt", [128, 2, C],
                           t_in["qw2dt"].ap().rearrange("(k p) m -> p k m", p=128),
                           bf16)
        eye_bf = load_param("eye_bf", [128, 128], t_in["eye_bf"].ap(), bf16)
        vecs = load_param("vecs", [128, 24],
                          t_in["vecs"].ap().rearrange("m p -> p m"))
        dws = load_param("dws", [128, 8, 9],
                         t_in["dws"].ap().rearrange("m p t -> p m t"))

        def vcol(base, i):
            return vecs[:, base + i:base + i + 1]

        # fixed rotating xpad buffers; zero the pad slots once
        xpads = []
        for i in range(N_XPAD_BUFS):
            xp = xpad_pool.tile([128, PADN], bf16, name=f"xpad{i}")
            nc.vector.memset(xp[:, 0:1], 0.0)
            zap = bass.AP(tensor=xp.tensor, offset=xp.offset + 49,
                          ap=[xp.ap[0], [WP, H], [1, 2]])
            nc.vector.memset(zap, 0.0)
            xpads.append(xp)

        data_view = lambda xp: bass.AP(
            tensor=xp.tensor, offset=xp.offset + 1,
            ap=[xp.ap[0], [WP, H], [1, W]])

        HALF_ROWS = 24
        CHUNKS = [(0, 10), (10, 10), (20, 4)]

        wt_tiles = {}      # (b, t) -> [128, 9] f32

        def pool_tile(b, t, bs_g, gi, engine="dve"):
            if b == 0 and t in xf_prefetch:
                xf = xf_prefetch[t]
            else:
                xf = xf32_pool.tile([128, HW], f32, name="xf")
                nc.sync.dma_start(
                    out=xf,
                    in_=x[b, 128 * t:128 * (t + 1)].rearrange(
                        "c h w -> c (h w)"))
            # bf16 padded cast on ScalarE (off the pooling critical path);
            # VectorE reduces straight from the f32 tile
            xp = xpads[(b * NT + t) % N_XPAD_BUFS]
            nc.scalar.activation(out=data_view(xp),
                                 in_=xf.rearrange("c (h w) -> c h w", h=H),
                                 func=AF.Copy)
            cb = cb_pool.tile([128, H, 3], f32, name="cb")
            nc.vector.reduce_sum(
                out=cb, in_=xf.rearrange("c (h j u) -> c h j u", j=3, u=16),
                axis=mybir.AxisListType.X)
            cb_r = bass.AP(tensor=cb.tensor, offset=cb.offset,
                           ap=[cb.ap[0], [3 * 16, 3], [1, 3], [3, 16]])
            nc.vector.reduce_sum(
                out=bs_g[t][:, 9 * gi:9 * gi + 9].rearrange(
                    "c (i j) -> c i j", i=3),
                in_=cb_r, axis=mybir.AxisListType.X)

        for g_idx, group in enumerate(GROUPS):
            G = len(group)
            # ---- pooling for all samples in the group ----
            # ScalarE takes part of the pooling (it is idle in these windows);
            # the last sample of each group goes first so its ACT chain and
            # the VectorE chain finish together.
            bs_g = bs_of[g_idx]
            for gi, b in enumerate(group):
                for t in range(NT):
                    pool_tile(b, t, bs_g, gi)

            # ---- generator branches, batched over the group ----
            N = 9 * G
            bs_bf = []
            for t in range(NT):
                c = sm_pool.tile([128, 9 * G], bf16, name="bs_bf", bufs=8)
                nc.vector.tensor_copy(out=c, in_=bs_g[t])
                bs_bf.append(c)

            h_bf = sm_pool.tile([128, 2, N], bf16, name="h_bf")
            for mc in range(2):
                hp = psum_small.tile([128, 32], f32, name="hp", tag="sp")
                for k in range(NT):
                    nc.tensor.matmul(hp[:, 0:N],
                                     lhsT=pw1t[:, k, 128 * mc:128 * (mc + 1)],
                                     rhs=bs_bf[k],
                                     start=(k == 0), stop=(k == NT - 1))
                erf_m = sm_pool.tile([128, N], f32, name="erf_m")
                nc.scalar.activation(out=erf_m, in_=hp[:, 0:N], func=AF.Erf,
                                     bias=vcol(V_PBETA_S, mc), scale=INV_SQRT2)
                z_m = sm_pool.tile([128, N], f32, name="z_m")
                nc.vector.tensor_scalar_add(out=z_m, in0=hp[:, 0:N],
                                            scalar1=vcol(V_PBETA, mc))
                t_m = sm_pool.tile([128, N], f32, name="t_m")
                nc.vector.tensor_scalar(out=t_m, in0=erf_m, scalar1=0.5,
                                        scalar2=0.5, op0=AL.mult, op1=AL.add)
                nc.vector.tensor_mul(out=h_bf[:, mc, :], in0=z_m, in1=t_m)

            hbv = sm_pool.tile([128, 2, G], f32, name="hbv")
            hb_bf = sm_pool.tile([128, 2, G], bf16, name="hb_bf")
            for mc in range(2):
                hq = psum_small.tile([128, 32], f32, name="hq", tag="sp")
                for k in range(NT):
                    nc.tensor.matmul(hq[:, 0:N],
                                     lhsT=qw1t[:, k, 128 * mc:128 * (mc + 1)],
                                     rhs=bs_bf[k],
                                     start=(k == 0), stop=(k == NT - 1))
                for gi in range(G):
                    nc.vector.reduce_sum(out=hbv[:, mc, gi:gi + 1],
                                         in_=hq[:, 9 * gi:9 * gi + 9],
                                         axis=mybir.AxisListType.X)
                erf_q = sm_pool.tile([128, G], f32, name="erf_q")
                nc.scalar.activation(out=erf_q, in_=hbv[:, mc, :], func=AF.Erf,
                                     bias=vcol(V_QBETA_S, mc), scale=INV_SQRT2)
                zq = sm_pool.tile([128, G], f32, name="zq")
                nc.vector.tensor_scalar_add(out=zq, in0=hbv[:, mc, :],
                                            scalar1=vcol(V_QBETA, mc))
                tq = sm_pool.tile([128, G], f32, name="tq")
                nc.vector.tensor_scalar(out=tq, in0=erf_q, scalar1=0.5,
                                        scalar2=0.5, op0=AL.mult, op1=AL.add)
                nc.vector.tensor_mul(out=hb_bf[:, mc, :], in0=zq, in1=tq)

            biasv = wt_pool.tile([128, NT, G], f32, name="biasv")
            for m in range(NT):
                sp = psum_small.tile([128, 32], f32, name="sp", tag="sp")
                nc.tensor.matmul(sp[:, 0:N],
                                 lhsT=pw2t[:, 0, 128 * m:128 * (m + 1)],
                                 rhs=h_bf[:, 0, :], start=True, stop=False)
                nc.tensor.matmul(sp[:, 0:N],
                                 lhsT=pw2t[:, 1, 128 * m:128 * (m + 1)],
                                 rhs=h_bf[:, 1, :], start=False, stop=True)
                tmp = sm_pool.tile([128, N], f32, name="tmp")
                nc.vector.tensor_scalar_mul(out=tmp, in0=bs_g[m],
                                            scalar1=1.0 / 256.0)
                sigarg = sm_pool.tile([128, N], f32, name="sigarg")
                nc.vector.tensor_tensor(out=sigarg, in0=sp[:, 0:N], in1=tmp,
                                        op=AL.subtract)
                s0 = sm_pool.tile([128, N], f32, name="s0")
                nc.scalar.activation(out=s0, in_=sigarg, func=AF.Sigmoid,
                                     bias=vcol(V_PB2, m))
                for gi, b in enumerate(group):
                    wt_t = wt_pool.tile([128, 9], f32, name="wt_t")
                    nc.vector.tensor_mul(out=wt_t,
                                         in0=s0[:, 9 * gi:9 * gi + 9],
                                         in1=dws[:, 4 + m, :])
                    nc.vector.tensor_add(out=wt_t, in0=wt_t, in1=dws[:, m, :])
                    wt_tiles[(b, m)] = wt_t

                zp = psum_small.tile([128, 32], f32, name="zp", tag="sp")
                nc.tensor.matmul(zp[:, 0:G],
                                 lhsT=qw2dt[:, 0, 128 * m:128 * (m + 1)],
                                 rhs=hb_bf[:, 0, :], start=True, stop=False)
                nc.tensor.matmul(zp[:, 0:G],
                                 lhsT=qw2dt[:, 1, 128 * m:128 * (m + 1)],
                                 rhs=hb_bf[:, 1, :], start=False, stop=True)
                bs0 = sm_pool.tile([128, G], f32, name="bs0")
                nc.scalar.activation(out=bs0, in_=zp[:, 0:G], func=AF.Sigmoid,
                                     bias=vcol(V_QB2D, m))
                nc.vector.tensor_scalar(out=biasv[:, m, :], in0=bs0,
                                        scalar1=vcol(V_DWBD, m),
                                        scalar2=vcol(V_DWB1, m),
                                        op0=AL.mult, op1=AL.add)

            # ---- depthwise conv for each sample in the group ----
            dve_tap = g_idx == 1      # tap (1,1) on VectorE for these samples
            taps_pe = TAPS[:-1] if dve_tap else TAPS
            for gi, b in enumerate(group):
                for t in range(NT):
                    xp = xpads[(b * NT + t) % N_XPAD_BUFS]
                    wt_t = wt_tiles[(b, t)]
                    dg_all = diag_pool.tile([128, 9, 128], bf16, name="dg_all")
                    eye9 = bass.AP(tensor=eye_bf.tensor, offset=eye_bf.offset,
                                   ap=[eye_bf.ap[0], [0, 9], [1, 128]])
                    wt_b = bass.AP(tensor=wt_t.tensor, offset=wt_t.offset,
                                   ap=[wt_t.ap[0], [1, 9], [0, 128]])
                    nc.vector.tensor_tensor(out=dg_all, in0=eye9, in1=wt_b,
                                            op=AL.mult)
                    diags = [dg_all[:, 3 * (dy + 1) + (dx + 1), :]
                             for (dy, dx) in TAPS]

                    ysb = ysb_pool.tile([128, HW], f32, name="ysb")
                    for half in range(2):
                        r_base = HALF_ROWS * half
                        pconv = psum_conv.tile([128, 3 * 512], f32,
                                               name="pconv")
                        for ti, (dy, dx) in enumerate(taps_pe):
                            first = ti == 0
                            last = ti == len(taps_pe) - 1
                            for j, (roff, nrows) in enumerate(CHUNKS):
                                r0 = r_base + roff
                                r_lo = max(r0, -dy)
                                r_hi = min(r0 + nrows, H - max(dy, 0))
                                if r_hi <= r_lo:
                                    continue
                                ncols = (r_hi - r_lo) * W
                                out_off = 512 * j + (r_lo - r0) * W
                                in_off = 1 + WP * (r_lo + dy) + dx
                                rhs = bass.AP(
                                    tensor=xp.tensor,
                                    offset=xp.offset + in_off,
                                    ap=[xp.ap[0], [WP, r_hi - r_lo], [1, W]])
                                nc.tensor.matmul(
                                    pconv[:, out_off:out_off + ncols],
                                    lhsT=diags[ti], rhs=rhs,
                                    start=first, stop=last,
                                    skip_group_check=True)
                        src01 = bass.AP(tensor=pconv.tensor,
                                        offset=pconv.offset,
                                        ap=[pconv.ap[0], [512, 2], [1, 480]])
                        nc.scalar.activation(
                            out=ysb[:, 1152 * half:1152 * half + 960],
                            in_=src01, func=AF.Identity,
                            bias=biasv[:, t, gi:gi + 1])
                        nc.scalar.activation(
                            out=ysb[:, 1152 * half + 960:1152 * half + 1152],
                            in_=pconv[:, 1024:1216], func=AF.Identity,
                            bias=biasv[:, t, gi:gi + 1])
                        if dve_tap:
                            # tap (dy=1, dx=1) applied on VectorE into ysb
                            r_lo = r_base
                            r_hi = min(r_base + HALF_ROWS, H - 1)
                            nr = r_hi - r_lo
                            nc11 = nr * W
                            iv = bass.AP(
                                tensor=xp.tensor,
                                offset=xp.offset + 1 + WP * (r_lo + 1) + 1,
                                ap=[xp.ap[0], [WP, nr], [1, W]])
                            tmp11 = sm_pool.tile([128, 1152], f32,
                                                 name="tmp11", bufs=4)
                            nc.vector.tensor_scalar_mul(
                                out=tmp11[:, 0:nc11], in0=iv,
                                scalar1=wt_tiles[(b, t)][:, 8:9])
                            nc.vector.tensor_tensor(
                                out=ysb[:, 1152 * half:1152 * half + nc11],
                                in0=ysb[:, 1152 * half:1152 * half + nc11],
                                in1=tmp11[:, 0:nc11], op=AL.add)
                        yv = y[b, 128 * t:128 * (t + 1)].rearrange(
                            "c h w -> c (h w)")
                        nc.sync.dma_start(
                            out=yv[:, 1152 * half:1152 * (half + 1)],
                            in_=ysb[:, 1152 * half:1152 * (half + 1)])



def _build():
    nc = bacc.Bacc("TRN2", debug=False, enable_asserts=False,
                   num_devices=N_CORES)
    t_in = {}
    def din(name, shape, dtype=f32):
        t_in[name] = nc.dram_tensor(name, list(shape), dtype,
                                    kind="ExternalInput")
    din("x", (BL, C, H, W))
    din("pw1t", (C, Cr), bf16)
    din("pw2t", (Cr, C), bf16)
    din("qw1t", (C, Cr), bf16)
    din("qw2dt", (Cr, C), bf16)
    din("eye_bf", (128, 128), bf16)
    din("vecs", (24, 128))
    din("dws", (8, 128, 9))
    t_out = {"y": nc.dram_tensor("y", [BL, C, H, W], f32,
                                 kind="ExternalOutput")}
    _emit_kernel(nc, t_in, t_out)
    nc.compile()
    return nc


_NC_CACHE = None


def _host_prep(inputs):
    f = np.float32
    bf = ml_dtypes.bfloat16
    p_inv = (inputs["p_bn_g"] / np.sqrt(inputs["p_bn_v"] + EPS)).astype(f)
    p_beta = (inputs["p_bn_b"] - inputs["p_bn_m"] * p_inv).astype(f)
    q_inv = (inputs["q_bn_g"] / np.sqrt(inputs["q_bn_v"] + EPS)).astype(f)
    q_beta = (inputs["q_bn_b"] - inputs["q_bn_m"] * q_inv).astype(f)
    dw = inputs["dw_weight"].reshape(2, C, 9).astype(f)
    dwb = inputs["dw_bias"].astype(f)
    vecs = np.concatenate([
        p_beta.reshape(2, 128),
        (p_beta * INV_SQRT2).reshape(2, 128),
        q_beta.reshape(2, 128),
        (q_beta * INV_SQRT2).reshape(2, 128),
        inputs["p_b2"].reshape(NT, 128).astype(f),
        (inputs["q_b2"][:C] - inputs["q_b2"][C:]).reshape(NT, 128).astype(f),
        dwb[1].reshape(NT, 128),
        (dwb[0] - dwb[1]).reshape(NT, 128),
    ], axis=0).astype(f)
    dws = np.concatenate([
        dw[1].reshape(NT, 128, 9),
        (dw[0] - dw[1]).reshape(NT, 128, 9),
    ], axis=0).astype(f)
    params = {
        "pw1t": np.ascontiguousarray(
            ((inputs["p_w1"] * p_inv[:, None]) / 256.0).T).astype(bf),
        "pw2t": np.ascontiguousarray(inputs["p_w2"].T).astype(bf),
        "qw1t": np.ascontiguousarray(
            ((inputs["q_w1"] * q_inv[:, None]) / 2304.0).T).astype(bf),
        "qw2dt": np.ascontiguousarray(
            (inputs["q_w2"][:C] - inputs["q_w2"][C:]).T).astype(bf),
        "eye_bf": np.eye(128).astype(bf),
        "vecs": vecs,
        "dws": dws,
    }
    return params


def kernel(**inputs):
    global _NC_CACHE
    if _NC_CACHE is None:
        _NC_CACHE = _build()
    nc = _NC_CACHE
    params = _host_prep(inputs)
    x = np.asarray(inputs["x"], dtype=np.float32)
    in_maps = []
    for i in range(N_CORES):
        m = dict(params)
        m["x"] = np.ascontiguousarray(x[BL * i:BL * (i + 1)])
        in_maps.append(m)
    res = run_bass_kernel_spmd(nc, in_maps, core_ids=list(range(N_CORES)))
    y = np.concatenate([res.results[i]["y"] for i in range(N_CORES)], axis=0)
    return y.astype(np.float32)


if __name__ == "__main__":
    nc = _build()
    print("build ok")


# revision 22
# speedup vs baseline: 1.0547x; 1.0547x over previous
"""Dynamic Depthwise Conv (DDC) module on 8 Trainium2 NeuronCores.

Strategy (data-parallel over batch, 4 samples/core):
  - Depthwise 3x3 conv on TensorE as 9 "diagonal matmuls" per 128-channel
    tile: lhsT = diag(per-channel tap weight) [128x128] bf16, rhs = a
    zero-column-padded bf16 image tile, accumulated in PSUM (fp32).
  - Kernel/bias generation branches (pooling -> 1x1 convs -> BN -> GELU ->
    1x1 conv -> softmax-over-2 == sigmoid of difference) computed on-chip:
    pooling block sums on VectorE, small matmuls on TensorE in bf16 batched
    over sample groups [1, 3] (group 0 unblocks the conv pipeline early,
    group 1 amortizes weight loads), exact GELU via Erf on ScalarE (all
    transcendentals live in the single `sigmoid_and_others` ACT table set).
  - BN scale and the 1/256 / 1/2304 pooling-mean factors are folded into
    host-precomputed transposed weight matrices (layout prep only).
  - f32->bf16 padded-image casts run on GpSimd to keep VectorE for pooling.
  - PSUM->SBUF copyback with per-channel bias add on ScalarE (Identity+bias).
"""

import numpy as np
import ml_dtypes
from contextlib import ExitStack

import concourse.bass as bass
import concourse.bacc as bacc
import concourse.tile as tile
import concourse.mybir as mybir
from concourse.bass_utils import run_bass_kernel_spmd

f32 = mybir.dt.float32
bf16 = mybir.dt.bfloat16
AF = mybir.ActivationFunctionType
AL = mybir.AluOpType

N_CORES = 8
B, C, H, W = 32, 512, 48, 48
BL = B // N_CORES          # samples per core
NT = C // 128              # channel tiles per sample
HW = H * W                 # 2304
WP = W + 2                 # padded row length (2 zero cols at end)
PADN = H * WP + 2          # 1 guard elem at 0, data at 1..; 2402
Cr = 256
EPS = 1e-5
INV_SQRT2 = 0.7071067811865476
GROUPS = [[0], [1], [2, 3]]  # sample batching for the generator branches

# tap order: dy=0 taps first so the first matmul per PSUM bank covers all rows
TAPS = [(0, 0), (0, -1), (0, 1),
        (-1, -1), (-1, 0), (-1, 1),
        (1, -1), (1, 0), (1, 1)]
N_XPAD_BUFS = 14

# vecs tile column layout
V_PBETA, V_PBETA_S, V_QBETA, V_QBETA_S = 0, 2, 4, 6
V_PB2, V_QB2D, V_DWB1, V_DWBD = 8, 12, 16, 20


def _emit_kernel(nc, t_in, t_out):
    x = t_in["x"].ap()            # [BL, C, H, W] f32
    y = t_out["y"].ap()           # [BL, C, H, W] f32

    with tile.TileContext(nc) as tc, ExitStack() as ctx:
        singles = ctx.enter_context(tc.tile_pool(name="singles", bufs=1))
        xf32_pool = ctx.enter_context(tc.tile_pool(name="xf32", bufs=4))
        xpad_pool = ctx.enter_context(tc.tile_pool(name="xpad", bufs=1))
        cb_pool = ctx.enter_context(tc.tile_pool(name="cb", bufs=4))
        bs_pool = ctx.enter_context(tc.tile_pool(name="bs", bufs=2 * NT))
        sm_pool = ctx.enter_context(tc.tile_pool(name="smalls", bufs=6))
        wt_pool = ctx.enter_context(tc.tile_pool(name="wt", bufs=2 * BL * NT))
        diag_pool = ctx.enter_context(tc.tile_pool(name="diag", bufs=4))
        ysb_pool = ctx.enter_context(tc.tile_pool(name="ysb", bufs=2))
        psum_small = ctx.enter_context(
            tc.tile_pool(name="ps_small", bufs=2, space="PSUM"))
        psum_conv = ctx.enter_context(
            tc.tile_pool(name="ps_conv", bufs=2, space="PSUM"))

        bs_of = {g: [bs_pool.tile([128, 9 * len(grp)], f32,
                                  name=f"bs_g{g}_{t}")
                     for t in range(NT)]
                 for g, grp in enumerate(GROUPS)}

        # prefetch sample-0 image tiles before the parameter DMAs so pooling
        # starts as early as possible
        xf_prefetch = {}
        for t in range(NT):
            xf = xf32_pool.tile([128, HW], f32, name="xf")
            nc.sync.dma_start(
                out=xf,
                in_=x[0, 128 * t:128 * (t + 1)].rearrange("c h w -> c (h w)"))
            xf_prefetch[t] = xf

        def load_param(name, sbuf_shape, src_ap, dtype=f32):
            t = singles.tile(sbuf_shape, dtype, name=name)
            nc.sync.dma_start(out=t, in_=src_ap)
            return t

        # big matrices in bf16 (matmul weights), vectors consolidated
        pw1t = load_param("pw1t", [128, NT, Cr],
                          t_in["pw1t"].ap().rearrange("(k p) m -> p k m", p=128),
                          bf16)
        pw2t = load_param("pw2t", [128, 2, C],
                          t_in["pw2t"].ap().rearrange("(k p) m -> p k m", p=128),
                          bf16)
        qw1t = load_param("qw1t", [128, NT, Cr],
                          t_in["qw1t"].ap().rearrange("(k p) m -> p k m", p=128),
                          bf16)
        qw2dt = load_param("qw2dt", [128, 2, C],
                           t_in["qw2dt"].ap().rearrange("(k p) m -> p k m", p=128),
                           bf16)
        eye_bf = load_param("eye_bf", [128, 128], t_in["eye_bf"].ap(), bf16)
        vecs = load_param("vecs", [128, 24],
                          t_in["vecs"].ap().rearrange("m p -> p m"))
        dws = load_param("dws", [128, 8, 9],
                         t_in["dws"].ap().rearrange("m p t -> p m t"))

        def vcol(base, i):
            return vecs[:, base + i:base + i + 1]

        # fixed rotating xpad buffers; zero the pad slots once
        xpads = []
        for i in range(N_XPAD_BUFS):
            xp = xpad_pool.tile([128, PADN], bf16, name=f"xpad{i}")
            nc.vector.memset(xp[:, 0:1], 0.0)
            zap = bass.AP(tensor=xp.tensor, offset=xp.offset + 49,
                          ap=[xp.ap[0], [WP, H], [1, 2]])
            nc.vector.memset(zap, 0.0)
            xpads.append(xp)

        data_view = lambda xp: bass.AP(
            tensor=xp.tensor, offset=xp.offset + 1,
            ap=[xp.ap[0], [WP, H], [1, W]])

        HALF_ROWS = 24
        CHUNKS = [(0, 10), (10, 10), (20, 4)]

        wt_tiles = {}      # (b, t) -> [128, 9] f32

        def pool_tile(b, t, bs_g, gi, engine="dve"):
            if b == 0 and t in xf_prefetch:
                xf = xf_prefetch[t]
            else:
                xf = xf32_pool.tile([128, HW], f32, name="xf")
                nc.sync.dma_start(
                    out=xf,
                    in_=x[b, 128 * t:128 * (t + 1)].rearrange(
                        "c h w -> c (h w)"))
            # bf16 padded cast on ScalarE (off the pooling critical path);
            # VectorE reduces straight from the f32 tile
            xp = xpads[(b * NT + t) % N_XPAD_BUFS]
            nc.scalar.activation(out=data_view(xp),
                                 in_=xf.rearrange("c (h w) -> c h w", h=H),
                                 func=AF.Copy)
            cb = cb_pool.tile([128, H, 3], f32, name="cb")
            nc.vector.reduce_sum(
                out=cb, in_=xf.rearrange("c (h j u) -> c h j u", j=3, u=16),
                axis=mybir.AxisListType.X)
            cb_r = bass.AP(tensor=cb.tensor, offset=cb.offset,
                           ap=[cb.ap[0], [3 * 16, 3], [1, 3], [3, 16]])
            nc.vector.reduce_sum(
                out=bs_g[t][:, 9 * gi:9 * gi + 9].rearrange(
                    "c (i j) -> c i j", i=3),
                in_=cb_r, axis=mybir.AxisListType.X)

        for g_idx, group in enumerate(GROUPS):
            G = len(group)
            # ---- pooling for all samples in the group ----
            # ScalarE takes part of the pooling (it is idle in these windows);
            # the last sample of each group goes first so its ACT chain and
            # the VectorE chain finish together.
            bs_g = bs_of[g_idx]
            for gi, b in enumerate(group):
                for t in range(NT):
                    pool_tile(b, t, bs_g, gi)

            # ---- generator branches, batched over the group ----
            N = 9 * G
            bs_bf = []
            for t in range(NT):
                c = sm_pool.tile([128, 9 * G], bf16, name="bs_bf", bufs=8)
                nc.vector.tensor_copy(out=c, in_=bs_g[t])
                bs_bf.append(c)

            h_bf = sm_pool.tile([128, 2, N], bf16, name="h_bf")
            for mc in range(2):
                hp = psum_small.tile([128, 32], f32, name="hp", tag="sp")
                for k in range(NT):
                    nc.tensor.matmul(hp[:, 0:N],
                                     lhsT=pw1t[:, k, 128 * mc:128 * (mc + 1)],
                                     rhs=bs_bf[k],
                                     start=(k == 0), stop=(k == NT - 1))
                erf_m = sm_pool.tile([128, N], f32, name="erf_m")
                nc.scalar.activation(out=erf_m, in_=hp[:, 0:N], func=AF.Erf,
                                     bias=vcol(V_PBETA_S, mc), scale=INV_SQRT2)
                z_m = sm_pool.tile([128, N], f32, name="z_m")
                nc.vector.tensor_scalar_add(out=z_m, in0=hp[:, 0:N],
                                            scalar1=vcol(V_PBETA, mc))
                t_m = sm_pool.tile([128, N], f32, name="t_m")
                nc.vector.tensor_scalar(out=t_m, in0=erf_m, scalar1=0.5,
                                        scalar2=0.5, op0=AL.mult, op1=AL.add)
                nc.vector.tensor_mul(out=h_bf[:, mc, :], in0=z_m, in1=t_m)

            hbv = sm_pool.tile([128, 2, G], f32, name="hbv")
            hb_bf = sm_pool.tile([128, 2, G], bf16, name="hb_bf")
            for mc in range(2):
                hq = psum_small.tile([128, 32], f32, name="hq", tag="sp")
                for k in range(NT):
                    nc.tensor.matmul(hq[:, 0:N],
                                     lhsT=qw1t[:, k, 128 * mc:128 * (mc + 1)],
                                     rhs=bs_bf[k],
                                     start=(k == 0), stop=(k == NT - 1))
                for gi in range(G):
                    nc.vector.reduce_sum(out=hbv[:, mc, gi:gi + 1],
                                         in_=hq[:, 9 * gi:9 * gi + 9],
                                         axis=mybir.AxisListType.X)
                erf_q = sm_pool.tile([128, G], f32, name="erf_q")
                nc.scalar.activation(out=erf_q, in_=hbv[:, mc, :], func=AF.Erf,
                                     bias=vcol(V_QBETA_S, mc), scale=INV_SQRT2)
                zq = sm_pool.tile([128, G], f32, name="zq")
                nc.vector.tensor_scalar_add(out=zq, in0=hbv[:, mc, :],
                                            scalar1=vcol(V_QBETA, mc))
                tq = sm_pool.tile([128, G], f32, name="tq")
                nc.vector.tensor_scalar(out=tq, in0=erf_q, scalar1=0.5,
                                        scalar2=0.5, op0=AL.mult, op1=AL.add)
                nc.vector.tensor_mul(out=hb_bf[:, mc, :], in0=zq, in1=tq)

            biasv = wt_pool.tile([128, NT, G], f32, name="biasv")
            for m in range(NT):
                sp = psum_small.tile([128, 32], f32, name="sp", tag="sp")
                nc.tensor.matmul(sp[:, 0:N],
                                 lhsT=pw2t[:, 0, 128 * m:128 * (m + 1)],
                                 rhs=h_bf[:, 0, :], start=True, stop=False)
                nc.tensor.matmul(sp[:, 0:N],
                                 lhsT=pw2t[:, 1, 128 * m:128 * (m + 1)],
                                 rhs=h_bf[:, 1, :], start=False, stop=True)
                tmp = sm_pool.tile([128, N], f32, name="tmp")
                nc.vector.tensor_scalar_mul(out=tmp, in0=bs_g[m],
                                            scalar1=1.0 / 256.0)
                sigarg = sm_pool.tile([128, N], f32, name="sigarg")
                nc.vector.tensor_tensor(out=sigarg, in0=sp[:, 0:N], in1=tmp,
                                        op=AL.subtract)
                s0 = sm_pool.tile([128, N], f32, name="s0")
                nc.scalar.activation(out=s0, in_=sigarg, func=AF.Sigmoid,
                                     bias=vcol(V_PB2, m))
                for gi, b in enumerate(group):
                    wt_t = wt_pool.tile([128, 9], f32, name="wt_t")
                    nc.vector.tensor_mul(out=wt_t,
                                         in0=s0[:, 9 * gi:9 * gi + 9],
                                         in1=dws[:, 4 + m, :])
                    nc.vector.tensor_add(out=wt_t, in0=wt_t, in1=dws[:, m, :])
                    wt_tiles[(b, m)] = wt_t

                zp = psum_small.tile([128, 32], f32, name="zp", tag="sp")
                nc.tensor.matmul(zp[:, 0:G],
                                 lhsT=qw2dt[:, 0, 128 * m:128 * (m + 1)],
                                 rhs=hb_bf[:, 0, :], start=True, stop=False)
                nc.tensor.matmul(zp[:, 0:G],
                                 lhsT=qw2dt[:, 1, 128 * m:128 * (m + 1)],
                                 rhs=hb_bf[:, 1, :], start=False, stop=True)
                bs0 = sm_pool.tile([128, G], f32, name="bs0")
                nc.scalar.activation(out=bs0, in_=zp[:, 0:G], func=AF.Sigmoid,
                                     bias=vcol(V_QB2D, m))
                nc.vector.tensor_scalar(out=biasv[:, m, :], in0=bs0,
                                        scalar1=vcol(V_DWBD, m),
                                        scalar2=vcol(V_DWB1, m),
                                        op0=AL.mult, op1=AL.add)

            # ---- depthwise conv for each sample in the group ----
            dve_tap = g_idx >= 1      # tap (1,1) on VectorE for these samples
            taps_pe = TAPS[:-1] if dve_tap else TAPS
            for gi, b in enumerate(group):
                for t in range(NT):
                    xp = xpads[(b * NT + t) % N_XPAD_BUFS]
                    wt_t = wt_tiles[(b, t)]
                    dg_all = diag_pool.tile([128, 9, 128], bf16, name="dg_all")
                    eye9 = bass.AP(tensor=eye_bf.tensor, offset=eye_bf.offset,
                                   ap=[eye_bf.ap[0], [0, 9], [1, 128]])
                    wt_b = bass.AP(tensor=wt_t.tensor, offset=wt_t.offset,
                                   ap=[wt_t.ap[0], [1, 9], [0, 128]])
                    nc.vector.tensor_tensor(out=dg_all, in0=eye9, in1=wt_b,
                                            op=AL.mult)
                    diags = [dg_all[:, 3 * (dy + 1) + (dx + 1), :]
                             for (dy, dx) in TAPS]

                    ysb = ysb_pool.tile([128, HW], f32, name="ysb")
                    for half in range(2):
                        r_base = HALF_ROWS * half
                        pconv = psum_conv.tile([128, 3 * 512], f32,
                                               name="pconv")
                        for ti, (dy, dx) in enumerate(taps_pe):
                            first = ti == 0
                            last = ti == len(taps_pe) - 1
                            for j, (roff, nrows) in enumerate(CHUNKS):
                                r0 = r_base + roff
                                r_lo = max(r0, -dy)
                                r_hi = min(r0 + nrows, H - max(dy, 0))
                                if r_hi <= r_lo:
                                    continue
                                ncols = (r_hi - r_lo) * W
                                out_off = 512 * j + (r_lo - r0) * W
                                in_off = 1 + WP * (r_lo + dy) + dx
                                rhs = bass.AP(
                                    tensor=xp.tensor,
                                    offset=xp.offset + in_off,
                                    ap=[xp.ap[0], [WP, r_hi - r_lo], [1, W]])
                                nc.tensor.matmul(
                                    pconv[:, out_off:out_off + ncols],
                                    lhsT=diags[ti], rhs=rhs,
                                    start=first, stop=last,
                                    skip_group_check=True)
                        src01 = bass.AP(tensor=pconv.tensor,
                                        offset=pconv.offset,
                                        ap=[pconv.ap[0], [512, 2], [1, 480]])
                        nc.scalar.activation(
                            out=ysb[:, 1152 * half:1152 * half + 960],
                            in_=src01, func=AF.Identity,
                            bias=biasv[:, t, gi:gi + 1])
                        nc.scalar.activation(
                            out=ysb[:, 1152 * half + 960:1152 * half + 1152],
                            in_=pconv[:, 1024:1216], func=AF.Identity,
                            bias=biasv[:, t, gi:gi + 1])
                        if dve_tap:
                            # tap (dy=1, dx=1) applied on VectorE into ysb
                            r_lo = r_base
                            r_hi = min(r_base + HALF_ROWS, H - 1)
                            nr = r_hi - r_lo
                            nc11 = nr * W
                            iv = bass.AP(
                                tensor=xp.tensor,
                                offset=xp.offset + 1 + WP * (r_lo + 1) + 1,
                                ap=[xp.ap[0], [WP, nr], [1, W]])
                            tmp11 = sm_pool.tile([128, 1152], f32,
                                                 name="tmp11", bufs=4)
                            nc.vector.tensor_scalar_mul(
                                out=tmp11[:, 0:nc11], in0=iv,
                                scalar1=wt_tiles[(b, t)][:, 8:9])
                            nc.vector.tensor_tensor(
                                out=ysb[:, 1152 * half:1152 * half + nc11],
                                in0=ysb[:, 1152 * half:1152 * half + nc11],
                                in1=tmp11[:, 0:nc11], op=AL.add)
                        yv = y[b, 128 * t:128 * (t + 1)].rearrange(
                            "c h w -> c (h w)")
                        nc.sync.dma_start(
                            out=yv[:, 1152 * half:1152 * (half + 1)],
                            in_=ysb[:, 1152 * half:1152 * (half + 1)])



def _build():
    nc = bacc.Bacc("TRN2", debug=False, enable_asserts=False,
                   num_devices=N_CORES)
    t_in = {}
    def din(name, shape, dtype=f32):
        t_in[name] = nc.dram_tensor(name, list(shape), dtype,
                                    kind="ExternalInput")
    din("x", (BL, C, H, W))
    din("pw1t", (C, Cr), bf16)
    din("pw2t", (Cr, C), bf16)
    din("qw1t", (C, Cr), bf16)
    din("qw2dt", (Cr, C), bf16)
    din("eye_bf", (128, 128), bf16)
    din("vecs", (24, 128))
    din("dws", (8, 128, 9))
    t_out = {"y": nc.dram_tensor("y", [BL, C, H, W], f32,
                                 kind="ExternalOutput")}
    _emit_kernel(nc, t_in, t_out)
    nc.compile()
    return nc


_NC_CACHE = None


def _host_prep(inputs):
    f = np.float32
    bf = ml_dtypes.bfloat16
    p_inv = (inputs["p_bn_g"] / np.sqrt(inputs["p_bn_v"] + EPS)).astype(f)
    p_beta = (inputs["p_bn_b"] - inputs["p_bn_m"] * p_inv).astype(f)
    q_inv = (inputs["q_bn_g"] / np.sqrt(inputs["q_bn_v"] + EPS)).astype(f)
    q_beta = (inputs["q_bn_b"] - inputs["q_bn_m"] * q_inv).astype(f)
    dw = inputs["dw_weight"].reshape(2, C, 9).astype(f)
    dwb = inputs["dw_bias"].astype(f)
    vecs = np.concatenate([
        p_beta.reshape(2, 128),
        (p_beta * INV_SQRT2).reshape(2, 128),
        q_beta.reshape(2, 128),
        (q_beta * INV_SQRT2).reshape(2, 128),
        inputs["p_b2"].reshape(NT, 128).astype(f),
        (inputs["q_b2"][:C] - inputs["q_b2"][C:]).reshape(NT, 128).astype(f),
        dwb[1].reshape(NT, 128),
        (dwb[0] - dwb[1]).reshape(NT, 128),
    ], axis=0).astype(f)
    dws = np.concatenate([
        dw[1].reshape(NT, 128, 9),
        (dw[0] - dw[1]).reshape(NT, 128, 9),
    ], axis=0).astype(f)
    params = {
        "pw1t": np.ascontiguousarray(
            ((inputs["p_w1"] * p_inv[:, None]) / 256.0).T).astype(bf),
        "pw2t": np.ascontiguousarray(inputs["p_w2"].T).astype(bf),
        "qw1t": np.ascontiguousarray(
            ((inputs["q_w1"] * q_inv[:, None]) / 2304.0).T).astype(bf),
        "qw2dt": np.ascontiguousarray(
            (inputs["q_w2"][:C] - inputs["q_w2"][C:]).T).astype(bf),
        "eye_bf": np.eye(128).astype(bf),
        "vecs": vecs,
        "dws": dws,
    }
    return params


def kernel(**inputs):
    global _NC_CACHE
    if _NC_CACHE is None:
        _NC_CACHE = _build()
    nc = _NC_CACHE
    params = _host_prep(inputs)
    x = np.asarray(inputs["x"], dtype=np.float32)
    in_maps = []
    for i in range(N_CORES):
        m = dict(params)
        m["x"] = np.ascontiguousarray(x[BL * i:BL * (i + 1)])
        in_maps.append(m)
    res = run_bass_kernel_spmd(nc, in_maps, core_ids=list(range(N_CORES)))
    y = np.concatenate([res.results[i]["y"] for i in range(N_CORES)], axis=0)
    return y.astype(np.float32)


if __name__ == "__main__":
    nc = _build()
    print("build ok")
